# revision 7
# baseline (speedup 1.0000x reference)
"""Cluster-memory cross-entropy loss on 8 Trainium2 NeuronCores.

Problem: loss = -mean_b log_softmax(normalize(inputs) @ features.T / T)[b, targets[b]]
  inputs [512, 256] f32, features [65536, 256] f32 (unit rows), targets [512] int.

Strategy: shard the 65536 cluster columns across 8 cores (8192 each). Each
core computes its shard's per-row sum(exp(logits)) with TensorE (f32r
matmuls) + ScalarE (fused exp + free-dim accumulation). The host combines
the 8 partial sum-exp vectors, computes log, and takes the target logits
with a trivial 512-row gather-dot (exact fp32).

Per-core input is a single [128, 17408] f32 blob laid out in SBUF order:
  cols [0, 1024):        xt[p, k, m]  = normalize(inputs)[m, k*128+p] / TEMP
  cols [1024 + jg*4096): ft[p, k, n]  = features[c*8192 + jg*2048 + n, k*128+p]
so every DMA is a contiguous per-partition range, and the first DMA carries
both xt and the first feature block (keeps matmul sync-wait count at 1).
"""

import numpy as np

import concourse.bass as bass
import concourse.mybir as mybir
import concourse.tile as tile
from concourse import bacc
from concourse.bass_utils import run_bass_kernel_spmd

B, N, D, TEMP = 512, 65536, 256, 0.05
NCORES = 8
NSH = N // NCORES      # 8192 cluster columns per core
JG = 4                 # column groups per core
JGW = NSH // JG        # 2048 columns per group (one PSUM tile)
MT = B // 128          # 4 batch tiles of 128
KT = D // 128          # 2 contraction chunks of 128
MM_N = 512             # moving free-dim per matmul

XT_W = KT * B          # 1024 cols of blob for xt
FT_W = KT * JGW        # 4096 cols of blob per feature group
BLOB_W = XT_W + JG * FT_W  # 17408

F32 = mybir.dt.float32
BF16 = mybir.dt.bfloat16

# f32r: fp32 data, PE runs it at full (1 cyc/row) rate with reduced-precision
# multiplies. Set to mybir.dt.float32 for the exact-but-4x-slower path.
MM_DTYPE = mybir.dt.float32r


def build_nc():
    nc = bacc.Bacc(target_bir_lowering=False)
    data = nc.declare_dram_parameter("data", [128, BLOB_W], MM_DTYPE, isOutput=False)
    out = nc.declare_dram_parameter("out", [128, MT * JG], F32, isOutput=True)

    with tile.TileContext(nc) as tc:
        with (
            tc.tile_pool(name="d0_pool", bufs=1) as d0_pool,
            tc.tile_pool(name="ft_pool", bufs=3) as ft_pool,
            tc.tile_pool(name="psum", bufs=2, space="PSUM") as psum_pool,
            tc.tile_pool(name="dead", bufs=2) as dead_pool,
            tc.tile_pool(name="acc", bufs=1) as acc_pool,
        ):
            # first chunk: xt + feature group 0, one DMA
            d0 = d0_pool.tile([128, XT_W + FT_W], MM_DTYPE)
            nc.sync.dma_start(out=d0[:], in_=data[:, 0 : XT_W + FT_W])
            xt_t = d0[:, 0:XT_W].rearrange("p (k b) -> p k b", k=KT)

            acc = acc_pool.tile([128, MT * JG], F32)

            for jg in range(JG):
                if jg == 0:
                    ft_ap = d0[:, XT_W : XT_W + FT_W]
                else:
                    ftt = ft_pool.tile([128, FT_W], MM_DTYPE)
                    nc.sync.dma_start(
                        out=ftt[:],
                        in_=data[:, XT_W + jg * FT_W : XT_W + (jg + 1) * FT_W],
                    )
                    ft_ap = ftt[:]
                ft3 = ft_ap.rearrange("p (k n) -> p k n", k=KT)

                for m in range(MT):
                    ps = psum_pool.tile([128, JGW], F32)
                    for k in range(KT):
                        for n in range(JGW // MM_N):
                            nc.tensor.matmul(
                                ps[:, n * MM_N:(n + 1) * MM_N],
                                lhsT=xt_t[:, k, m * 128:(m + 1) * 128],
                                rhs=ft3[:, k, n * MM_N:(n + 1) * MM_N],
                                start=(k == 0),
                                stop=(k == KT - 1),
                            )
                    deadt = dead_pool.tile([128, JGW], BF16)
                    nc.scalar.activation(
                        deadt[:],
                        ps[:],
                        mybir.ActivationFunctionType.Exp,
                        accum_out=acc[:, m * JG + jg : m * JG + jg + 1],
                    )
            nc.sync.dma_start(out=out[:], in_=acc[:])
    nc.compile()
    return nc


_NC_CACHE = {}


def _get_nc():
    if "nc" not in _NC_CACHE:
        _NC_CACHE["nc"] = build_nc()
    return _NC_CACHE["nc"]


def prep_inputs(inputs, features):
    """Host-side data prep: normalize+scale x, transpose both into the
    SBUF-resident layouts, pack per-core blobs so every DMA is contiguous."""
    xn = inputs / np.linalg.norm(inputs, axis=1, keepdims=True)
    xs = (xn / TEMP).astype(np.float32)
    # xt[p, k, m] = xs[m, k*128+p] -> flat [128, 1024]
    xt_flat = xs.reshape(B, KT, 128).transpose(2, 1, 0).reshape(128, XT_W)
    blobs = []
    for c in range(NCORES):
        fc = features[c * NSH:(c + 1) * NSH]  # [8192, 256]
        # ft[jg, p, k, n] = fc[jg*2048+n, k*128+p] -> [128, jg*k*n]
        ft_flat = (
            fc.reshape(JG, JGW, KT, 128).transpose(3, 0, 2, 1).reshape(128, JG * FT_W)
        )
        blob = np.empty((128, BLOB_W), dtype=np.float32)
        blob[:, :XT_W] = xt_flat
        blob[:, XT_W:] = ft_flat
        blobs.append(blob)
    return xs, blobs


def run_cores(blobs, **kwargs):
    nc = _get_nc()
    in_maps = [{"data": blobs[c]} for c in range(NCORES)]
    return run_bass_kernel_spmd(nc, in_maps, list(range(NCORES)), **kwargs)


def combine(results, xs, features, targets):
    # per-core out[p, m*JG+jg]: partial sum-exp of batch row m*128+p over
    # that core's jg-th 2048-column group.
    sumexp = np.zeros(B, dtype=np.float64)
    for c in range(NCORES):
        o = results[c]["out"].astype(np.float64)  # [128, 16]
        per_row = o.reshape(128, MT, JG).sum(axis=2)  # [128 (p), 4 (m)]
        sumexp += per_row.T.reshape(B)
    logz = np.log(sumexp)
    t_logit = (xs * features[targets]).sum(axis=1).astype(np.float64)
    loss = np.mean(logz - t_logit)
    return np.float32(loss)


def kernel(inputs, ema_inputs, targets, features):
    inputs = np.asarray(inputs, dtype=np.float32)
    features = np.asarray(features, dtype=np.float32)
    targets = np.asarray(targets)
    xs, blobs = prep_inputs(inputs, features)
    results = run_cores(blobs).results
    return combine(results, xs, features, targets)


# revision 8
# speedup vs baseline: 1.0071x; 1.0071x over previous
"""Cluster-memory cross-entropy loss on 8 Trainium2 NeuronCores.

Problem: loss = -mean_b log_softmax(normalize(inputs) @ features.T / T)[b, targets[b]]
  inputs [512, 256] f32, features [65536, 256] f32 (unit rows), targets [512] int.

Strategy: shard the 65536 cluster columns across 8 cores (8192 each). Each
core computes its shard's per-row sum(exp(logits)):
  TensorE: logit tiles [128 batch, 2048 clusters] in PSUM (f32r matmuls)
  ScalarE: exp PSUM -> SBUF
  VectorE: free-dim reduce -> per-tile partial sums
The host combines the 8 partial sum-exp vectors, computes log, and takes
the target logits with a trivial 512-row gather-dot (exact fp32).

Per-core input is a single [128, 17408] f32 blob laid out in SBUF order:
  cols [0, 1024):  xt[p, (k, m)]             = normalize(inputs)[m, k*128+p] / TEMP
  then per jg (4): [nch (4), k (2), n (512)] = features[c*8192 + jg*2048 + nch*512 + n, k*128+p]
so every DMA is a contiguous per-partition range: one 512 KB DMA for xt and
sixteen 512 KB slab DMAs, letting compute start after the first slab lands.
"""

import numpy as np

import concourse.bass as bass
import concourse.mybir as mybir
import concourse.tile as tile
from concourse import bacc
from concourse.bass_utils import run_bass_kernel_spmd

B, N, D, TEMP = 512, 65536, 256, 0.05
NCORES = 8
NSH = N // NCORES      # 8192 cluster columns per core
JG = 4                 # column groups per core (one PSUM tile each)
JGW = NSH // JG        # 2048 columns per group
NCH = 4                # 512-col slabs per group
MT = B // 128          # 4 batch tiles of 128
KT = D // 128          # 2 contraction chunks of 128
MM_N = JGW // NCH      # 512 moving free-dim per matmul

XT_W = KT * B          # 1024 blob cols for xt
SLAB_W = KT * MM_N     # 1024 blob cols per slab
BLOB_W = XT_W + JG * NCH * SLAB_W  # 17408

F32 = mybir.dt.float32
BF16 = mybir.dt.bfloat16

# f32r: fp32 bits, PE multiplies at reduced precision but ~2x the fp32 rate.
# Set to mybir.dt.float32 for the exact-but-slower path.
MM_DTYPE = mybir.dt.float32r


def build_nc():
    nc = bacc.Bacc(target_bir_lowering=False)
    data = nc.declare_dram_parameter("data", [128, BLOB_W], MM_DTYPE, isOutput=False)
    out = nc.declare_dram_parameter("out", [128, MT * JG], F32, isOutput=True)

    with tile.TileContext(nc) as tc:
        with (
            tc.tile_pool(name="xt_pool", bufs=1) as xt_pool,
            tc.tile_pool(name="slab_pool", bufs=10) as slab_pool,
            tc.tile_pool(name="psum", bufs=2, space="PSUM") as psum_pool,
            tc.tile_pool(name="expv", bufs=3) as exp_pool,
            tc.tile_pool(name="acc", bufs=1) as acc_pool,
        ):
            xt_t = xt_pool.tile([128, KT, B], MM_DTYPE)
            nc.sync.dma_start(
                out=xt_t[:], in_=data[:, 0:XT_W].rearrange("p (k b) -> p k b", k=KT)
            )
            acc = acc_pool.tile([128, MT * JG], F32)

            # prefetch queue of slab DMAs, issued in consumption order
            slabs = []
            for jg in range(JG):
                for nch in range(NCH):
                    st = slab_pool.tile([128, KT, MM_N], MM_DTYPE)
                    off = XT_W + (jg * NCH + nch) * SLAB_W
                    nc.sync.dma_start(
                        out=st[:],
                        in_=data[:, off : off + SLAB_W].rearrange(
                            "p (k n) -> p k n", k=KT
                        ),
                    )
                    slabs.append(st)

            for jg in range(JG):
                for m in range(MT):
                    ps = psum_pool.tile([128, JGW], F32)
                    for nch in range(NCH):
                        st = slabs[jg * NCH + nch]
                        for k in range(KT):
                            nc.tensor.matmul(
                                ps[:, nch * MM_N:(nch + 1) * MM_N],
                                lhsT=xt_t[:, k, m * 128:(m + 1) * 128],
                                rhs=st[:, k, :],
                                start=(k == 0),
                                stop=(k == KT - 1),
                            )
                    ev = exp_pool.tile([128, JGW], F32)
                    nc.scalar.activation(
                        ev[:], ps[:], mybir.ActivationFunctionType.Exp
                    )
                    nc.vector.reduce_sum(
                        acc[:, m * JG + jg : m * JG + jg + 1],
                        ev[:],
                        axis=mybir.AxisListType.X,
                    )
            nc.sync.dma_start(out=out[:], in_=acc[:])
    nc.compile()
    return nc


_NC_CACHE = {}


def _get_nc():
    if "nc" not in _NC_CACHE:
        _NC_CACHE["nc"] = build_nc()
    return _NC_CACHE["nc"]


def prep_inputs(inputs, features):
    """Host-side data prep: normalize+scale x, transpose both into the
    SBUF-resident layouts, pack per-core blobs so every DMA is contiguous."""
    xn = inputs / np.linalg.norm(inputs, axis=1, keepdims=True)
    xs = (xn / TEMP).astype(np.float32)
    # xt[p, k, m] = xs[m, k*128+p] -> flat [128, 1024]
    xt_flat = xs.reshape(B, KT, 128).transpose(2, 1, 0).reshape(128, XT_W)
    blobs = []
    for c in range(NCORES):
        fc = features[c * NSH:(c + 1) * NSH]  # [8192, 256]
        # slab[(jg,nch), p, k, n] = fc[(jg*4+nch)*512 + n, k*128+p]
        ft_flat = (
            fc.reshape(JG * NCH, MM_N, KT, 128)
            .transpose(3, 0, 2, 1)
            .reshape(128, JG * NCH * SLAB_W)
        )
        blob = np.empty((128, BLOB_W), dtype=np.float32)
        blob[:, :XT_W] = xt_flat
        blob[:, XT_W:] = ft_flat
        blobs.append(blob)
    return xs, blobs


def run_cores(blobs, **kwargs):
    nc = _get_nc()
    in_maps = [{"data": blobs[c]} for c in range(NCORES)]
    return run_bass_kernel_spmd(nc, in_maps, list(range(NCORES)), **kwargs)


def combine(results, xs, features, targets):
    # per-core out[p, m*JG+jg]: partial sum-exp of batch row m*128+p over
    # that core's jg-th 2048-column group.
    sumexp = np.zeros(B, dtype=np.float64)
    for c in range(NCORES):
        o = results[c]["out"].astype(np.float64)  # [128, 16]
        per_row = o.reshape(128, MT, JG).sum(axis=2)  # [128 (p), 4 (m)]
        sumexp += per_row.T.reshape(B)
    logz = np.log(sumexp)
    t_logit = (xs * features[targets]).sum(axis=1).astype(np.float64)
    loss = np.mean(logz - t_logit)
    return np.float32(loss)


def kernel(inputs, ema_inputs, targets, features):
    inputs = np.asarray(inputs, dtype=np.float32)
    features = np.asarray(features, dtype=np.float32)
    targets = np.asarray(targets)
    xs, blobs = prep_inputs(inputs, features)
    results = run_cores(blobs).results
    return combine(results, xs, features, targets)
